# revision 1
# baseline (speedup 1.0000x reference)
"""AttentionPool2d Trainium2 kernel, 8-core batch-data-parallel.

Math (reference returns only query position 0):
  xf = [x.flat, mean] + pos  (permuted: cols 0..255 spatial, col 256 = mean tok)
  q0 = W_q @ xf_m + b_q                 (the only query needed)
  u_h = W_k_h^T q0_h  (folds W_k into the query; k never materialized)
  logits = (1/8) u^T xf ; w = softmax(logits)
  y = xf @ w'^T (+ pos-terms)           (w' = w_sp + w_m/256 absorbs mean token)
  a0_h = W_v_h y_h + b_v ; out = w_c a0 + b_c
"""
import sys, types
sys.path.insert(0, "/opt/trn_rl_repo")
import numpy as np
import ml_dtypes
from contextlib import ExitStack

from concourse import bacc, tile, mybir
import concourse.bass as bass
from concourse import masks
from concourse.bass_utils import run_bass_kernel_spmd

P = 128
B, C, S2, L = 64, 1024, 256, 257
NH, CHD = 16, 64
NCORE, BPC, CT = 8, 8, 8          # cores, batches/core, c-tiles
F32R = mybir.dt.float32r
F32 = mybir.dt.float32
BF16 = mybir.dt.bfloat16
AF = mybir.ActivationFunctionType
SCALE2 = 1.0 / 8.0                 # (1/ch^0.25)^2 folded into u


def _body(ctx: ExitStack, tc, d):
    nc = tc.nc
    const = ctx.enter_context(tc.tile_pool(name="const", bufs=1))
    wbig = ctx.enter_context(tc.tile_pool(name="wbig", bufs=2))
    wsml = ctx.enter_context(tc.tile_pool(name="wsml", bufs=1))
    xres = ctx.enter_context(tc.tile_pool(name="xres", bufs=1))
    xtp = ctx.enter_context(tc.tile_pool(name="xtp", bufs=1))
    wbf = ctx.enter_context(tc.tile_pool(name="wbf", bufs=2))
    work = ctx.enter_context(tc.tile_pool(name="work", bufs=1))
    acc = ctx.enter_context(tc.tile_pool(name="acc", bufs=1))
    ps = ctx.enter_context(tc.tile_pool(name="ps", bufs=2, space="PSUM"))
    ps1 = ctx.enter_context(tc.tile_pool(name="ps1", bufs=2, space="PSUM"))

    identf = const.tile([16, 16], F32)
    masks.make_identity(nc, identf[:])
    ident = const.tile([16, 16], F32R)
    nc.vector.tensor_copy(ident[:], identf[:, :])

    # ---- stage A: x in, means, xf0 ----
    xs = []
    sums = acc.tile([P, BPC * CT], F32R)
    xf0 = acc.tile([P, BPC * CT], BF16)             # mean-token cols (b, j)
    scratch = work.tile([P, S2], F32R, tag="scr")
    xpairs = []
    for pr in range(BPC // 2):
        xp2 = xres.tile([P, 2, CT, S2 + 2], BF16, tag=f"xp{pr}")
        nc.sync.dma_start(
            xp2[:, :, :, 0:S2],
            d["x"].ap()[2 * pr:2 * pr + 2].rearrange(
                "b (j p) s -> p (b j) s", p=P).rearrange(
                "p (b j) s -> p b j s", b=2))
        nc.vector.tensor_scalar_mul(xp2[:, :, :, S2 + 1:S2 + 2],
                                     xp2[:, :, :, 0:1], 0.0)
        xpairs.append(xp2)
    for b in range(BPC):
        xb = xpairs[b // 2][:, b % 2]
        xs.append(xb)

    # ---- weights needed early (after x DMAs in queue order) ----
    wqt = wbig.tile([P, CT, C], BF16, tag="wbig")   # W_q^T  (c-part, q)
    nc.sync.dma_start(wqt[:], d["wqt"].ap().rearrange("(j p) q -> p j q", p=P))
    wk = wbig.tile([P, CT, C], BF16, tag="wbig")    # W_k natural (krow-part, c)
    nc.sync.dma_start(wk[:], d["wk"].ap().rearrange("(t k) c -> k t c", k=P))
    posn = wsml.tile([P, CT, L], BF16)              # permuted pos, natural
    nc.sync.dma_start(posn[:], d["posn"].ap().rearrange("(j p) s -> p j s", p=P))
    post = wsml.tile([P, 2, C], BF16)               # spatial pos, transposed
    nc.sync.dma_start(post[:], d["post"].ap().rearrange("(t p) c -> p t c", p=P))
    posc = wsml.tile([1, C], BF16)                  # pos0 - mean_sp(pos)
    nc.sync.dma_start(posc[:], d["posc"].ap())
    bq = wsml.tile([P, CT], F32R)
    nc.sync.dma_start(bq[:], d["bq"].ap().rearrange("(j p) -> p j", p=P))
    bv = wsml.tile([P, CT], F32R)
    nc.sync.dma_start(bv[:], d["bv"].ap().rearrange("(j p) -> p j", p=P))
    bcn = wsml.tile([P, CT], F32R)
    nc.sync.dma_start(bcn[:], d["bc"].ap().rearrange("(j p) -> p j", p=P))
    wvt = wbf.tile([P, CT, C], BF16, tag="wv")      # W_v^T (c-part, vch)
    nc.sync.dma_start(wvt[:], d["wvt"].ap().rearrange("(j p) v -> p j v", p=P))
    wct = wbf.tile([P, CT, C], BF16, tag="wc")      # w_c^T (vch-part, o)
    nc.sync.dma_start(wct[:], d["wct"].ap().rearrange("(r p) o -> p r o", p=P))

    for b in range(BPC):
        xb = xs[b]
        for j in range(CT):
            if j % 2 == 0:
                nc.vector.reduce_sum(sums[:, b * CT + j:b * CT + j + 1],
                                     xb[:, j, 0:S2], axis=mybir.AxisListType.X)
            else:
                nc.scalar.activation(scratch[:], xb[:, j, 0:S2], AF.Copy,
                                     accum_out=sums[:, b * CT + j:b * CT + j + 1])
        for j in range(CT):
            nc.scalar.activation(xf0[:, b * CT + j:b * CT + j + 1],
                                 sums[:, b * CT + j:b * CT + j + 1], AF.Identity,
                                 bias=posn[:, j, S2:S2 + 1], scale=1.0 / S2)
            nc.scalar.activation(xb[:, j, S2:S2 + 1],
                                 sums[:, b * CT + j:b * CT + j + 1], AF.Identity,
                                 bias=posn[:, j, S2:S2 + 1], scale=1.0 / S2)

    # ---- stage B: q0 (batched over b) ----
    q0f = ps1.tile([P, P], F32, tag="seq")
    q0p = q0f[:, 0:CT * BPC]        # (q-part, (i, b))
    for i in range(CT):
        for j in range(CT):
            nc.tensor.matmul(q0p[:, i * BPC:(i + 1) * BPC],
                             wqt[:, j, i * P:(i + 1) * P],
                             xf0[:, b0j(j)],
                             start=(j == 0), stop=(j == CT - 1))
    # block-diagonal q0 (+bias) for the per-head W_k^T fold
    q0blk = acc.tile([P, CT * 16], BF16)
    nc.vector.memset(q0blk[:], 0.0)
    for i in range(CT):
        nc.scalar.activation(q0blk[0:64, i * 16:i * 16 + 8],
                             q0p[0:64, i * BPC:i * BPC + 8], AF.Identity,
                             bias=bq[0:64, i:i + 1])
        nc.scalar.activation(q0blk[64:P, i * 16 + 8:i * 16 + 16],
                             q0p[64:P, i * BPC:i * BPC + 8], AF.Identity,
                             bias=bq[64:P, i:i + 1])

    # ---- stage C: u = blockdiag(W_k)^T q0, scaled ----
    usb = acc.tile([P, CT * P], BF16)               # (c-part, (j, h, b))
    for j in range(CT):
        up = ps1.tile([P, P], F32, tag="seq")
        for t in range(CT):
            nc.tensor.matmul(up[:, t * 16:(t + 1) * 16],
                             wk[:, t, j * P:(j + 1) * P],
                             q0blk[:, t * 16:(t + 1) * 16])
        nc.vector.tensor_scalar_mul(usb[:, j * P:(j + 1) * P], up[:, :], SCALE2)

    # ---- per-batch: logits, softmax, w' transposes, y_x ----
    xtall = xtp.tile([P, 2 * BPC, C], BF16)
    nc.sync.dma_start(xtall[:], d["xt"].ap().rearrange(
        "b (t p) c -> p (b t) c", p=P))
    wta = acc.tile([P, 3 * P], BF16)                # w'^T batched (s-part,(t,h,b))
    yall = acc.tile([P, CT * P], BF16)              # y (c-part, (j, h, b))
    ypsb = acc.tile([P, CT * P], BF16)              # y_pos (c-part, (j, h, b))
    for b in range(BPC):
        lg = ps.tile([16, S2 + 2], F32, tag="lg")
        ub = [usb[:, j * P + b: (j + 1) * P: 8] for j in range(CT)]
        for j in range(CT):
            nc.tensor.matmul(lg[:, 0:S2 + 2], ub[j], xs[b][:, j, :],
                             start=(j == 0), stop=False)
        for j in range(CT):
            nc.tensor.matmul(lg[:, 0:S2], ub[j], posn[:, j, 0:S2],
                             start=False, stop=(j == CT - 1))
        # softmax over 257
        mx = work.tile([16, 4], F32, tag="mx")
        nc.vector.reduce_max(mx[:, 0:1], lg[:, 0:L], axis=mybir.AxisListType.X,
                             negate=True)
        ex = work.tile([16, L], F32R, tag="ex")
        nc.scalar.activation(ex[:, :], lg[:, 0:L], AF.Exp, bias=mx[:, 0:1],
                             accum_out=mx[:, 1:2])
        nc.vector.reciprocal(mx[:, 2:3], mx[:, 1:2])
        # w' = (e_sp + e_m/256) * r ; wm = e_m * r
        wp = work.tile([16, L], F32R, tag="wp")
        nc.vector.tensor_scalar_mul(mx[:, 3:4], ex[:, S2:S2 + 1], 1.0 / S2)
        nc.vector.tensor_scalar(wp[:, 0:S2], ex[:, 0:S2], mx[:, 3:4], mx[:, 2:3],
                                op0=mybir.AluOpType.add,
                                op1=mybir.AluOpType.mult)
        nc.vector.tensor_scalar(wp[:, S2:L], ex[:, S2:L], mx[:, 2:3], None,
                                op0=mybir.AluOpType.mult)
        # transpose w' -> (s-part, h) chunks; third chunk = wm row
        wtp = ps.tile([P, 48], F32R, tag="wt")
        nc.tensor.transpose(wtp[:, 0:16], wp[:, 0:P],
                            ident[:, :])
        nc.tensor.transpose(wtp[:, 16:32], wp[:, P:S2],
                            ident[:, :])
        nc.tensor.transpose(wtp[0:1, 32:48], wp[:, S2:L],
                            ident[:, :])
        for t in range(2):
            nc.vector.tensor_copy(wta[:, t * P + b:(t + 1) * P:8],
                                  wtp[:, t * 16:(t + 1) * 16])
        nc.vector.tensor_copy(wta[0:1, 2 * P + b:3 * P:8], wtp[0:1, 32:48])
        # y_x: stationary x^T tiles, moving w'^T
        yp = ps.tile([P, P], F32, tag="y")
        for j in range(CT):
            for t in range(2):
                nc.tensor.matmul(yp[:, j * 16:(j + 1) * 16],
                                 xtall[:, 2 * b + t, j * P:(j + 1) * P],
                                 wta[:, t * P + b:(t + 1) * P:8],
                                 start=(t == 0), stop=(t == 1))
        # scatter y_b into (j, h, b) layout: stride-8 columns for batch b
        nc.vector.tensor_copy(yall[:, b::8], yp[:, :])

    # ---- y_pos batched: pos^T against all-b w'^T ----
    for j in range(CT):
        ypp = ps1.tile([P, P], F32, tag="seq")
        for t in range(2):
            nc.tensor.matmul(ypp[:, :], post[:, t, j * P:(j + 1) * P],
                             wta[:, t * P:(t + 1) * P], start=(t == 0), stop=False)
        nc.tensor.matmul(ypp[:, :], posc[0:1, j * P:(j + 1) * P],
                         wta[0:1, 2 * P:3 * P], start=False, stop=True)
        nc.vector.tensor_copy(ypsb[:, j * P:(j + 1) * P], ypp[:, :])
    yfin = acc.tile([P, CT * P], BF16)
    nc.vector.tensor_add(yfin[:, :], yall[:, :], ypsb[:, :])

    # ---- a0 = blockdiag(W_v) y  (+ b_v) ----
    a0p = ps1.tile([P, P], F32, tag="seq")
    for r in range(CT):
        for j in range(CT):
            nc.tensor.matmul(a0p[:, r * 16:(r + 1) * 16],
                             wvt[:, j, r * P:(r + 1) * P],
                             yfin[:, j * P + 2 * r * 8: j * P + 2 * r * 8 + 16],
                             start=(j == 0), stop=(j == CT - 1))
    a0 = acc.tile([P, CT * BPC], BF16)              # (vch-part, (r, b))
    for r in range(CT):
        nc.scalar.activation(a0[0:64, r * 8:(r + 1) * 8],
                             a0p[0:64, r * 16:r * 16 + 8], AF.Identity,
                             bias=bv[0:64, r:r + 1])
        nc.scalar.activation(a0[64:P, r * 8:(r + 1) * 8],
                             a0p[64:P, r * 16 + 8:(r + 1) * 16], AF.Identity,
                             bias=bv[64:P, r:r + 1])

    # ---- out = w_c a0 + b_c ----
    opf = ps1.tile([P, P], F32, tag="seq")
    op = opf[:, 0:CT * BPC]
    for i in range(CT):
        for r in range(CT):
            nc.tensor.matmul(op[:, i * BPC:(i + 1) * BPC],
                             wct[:, r, i * P:(i + 1) * P],
                             a0[:, r * BPC:(r + 1) * BPC],
                             start=(r == 0), stop=(r == CT - 1))
    osb = acc.tile([P, CT * BPC], F32)
    for i in range(CT):
        nc.scalar.activation(osb[:, i * BPC:(i + 1) * BPC],
                             op[:, i * BPC:(i + 1) * BPC], AF.Identity,
                             bias=bcn[:, i:i + 1])
    nc.sync.dma_start(d["out"].ap(), osb[:])


def b0j(j):
    # xf0 columns for all b at fixed j: (b, j) layout -> stride CT
    return slice(j, BPC * CT, CT)


_CACHE = {}


def _get_nc():
    if "nc" in _CACHE:
        return _CACHE["nc"]
    nc = bacc.Bacc("TRN2", target_bir_lowering=False, debug=False,
                   num_devices=NCORE)
    d = {}
    d["x"] = nc.dram_tensor("x", [BPC, C, S2], BF16, kind="ExternalInput")
    d["xt"] = nc.dram_tensor("xt", [BPC, S2, C], BF16, kind="ExternalInput")
    d["posn"] = nc.dram_tensor("posn", [C, L], BF16, kind="ExternalInput")
    d["post"] = nc.dram_tensor("post", [S2, C], BF16, kind="ExternalInput")
    d["posc"] = nc.dram_tensor("posc", [1, C], BF16, kind="ExternalInput")
    d["wqt"] = nc.dram_tensor("wqt", [C, C], BF16, kind="ExternalInput")
    d["wk"] = nc.dram_tensor("wk", [C, C], BF16, kind="ExternalInput")
    d["wvt"] = nc.dram_tensor("wvt", [C, C], BF16, kind="ExternalInput")
    d["wct"] = nc.dram_tensor("wct", [C, C], BF16, kind="ExternalInput")
    d["bq"] = nc.dram_tensor("bq", [C], F32R, kind="ExternalInput")
    d["bv"] = nc.dram_tensor("bv", [C], F32R, kind="ExternalInput")
    d["bc"] = nc.dram_tensor("bc", [C], F32R, kind="ExternalInput")
    d["out"] = nc.dram_tensor("out", [P, CT * BPC], F32, kind="ExternalOutput")
    with tile.TileContext(nc) as tc, ExitStack() as ctx, \
            nc.allow_low_precision(reason="float32r tiles hold f32 bits"):
        _body(ctx, tc, d)
    nc.compile()
    _CACHE["nc"] = nc
    return nc


def _prep_maps(inputs):
    xf32 = inputs["x"].reshape(B, C, S2).astype(np.float32)
    x = np.ascontiguousarray(xf32).astype(ml_dtypes.bfloat16)
    xt = np.ascontiguousarray(xf32.transpose(0, 2, 1)).astype(ml_dtypes.bfloat16)
    pos = inputs["pos_emb"].astype(np.float32)
    posn = np.ascontiguousarray(np.concatenate([pos[:, 1:], pos[:, :1]], axis=1)).astype(ml_dtypes.bfloat16)
    post = np.ascontiguousarray(pos[:, 1:].T).astype(ml_dtypes.bfloat16)
    posc = np.ascontiguousarray((pos[:, 0] - pos[:, 1:].mean(axis=1))[None, :]
                                ).astype(ml_dtypes.bfloat16)
    wqkv = inputs["w_qkv"].astype(np.float32)
    wqt = np.ascontiguousarray(wqkv[0:C].T).astype(ml_dtypes.bfloat16)
    wk = np.ascontiguousarray(wqkv[C:2 * C]).astype(ml_dtypes.bfloat16)
    wvt = np.ascontiguousarray(wqkv[2 * C:3 * C].T).astype(ml_dtypes.bfloat16)
    wct = np.ascontiguousarray(inputs["w_c"].astype(np.float32).T).astype(ml_dtypes.bfloat16)
    bqkv = inputs["b_qkv"].astype(np.float32)
    shared = dict(posn=posn, post=post, posc=posc, wqt=wqt, wk=wk, wvt=wvt,
                  wct=wct, bq=np.ascontiguousarray(bqkv[0:C]),
                  bv=np.ascontiguousarray(bqkv[2 * C:3 * C]),
                  bc=inputs["b_c"].astype(np.float32))
    maps = []
    for c in range(NCORE):
        m = dict(shared)
        m["x"] = np.ascontiguousarray(x[c * BPC:(c + 1) * BPC])
        m["xt"] = np.ascontiguousarray(xt[c * BPC:(c + 1) * BPC])
        maps.append(m)
    return maps


def kernel(**inputs) -> np.ndarray:
    nc = _get_nc()
    maps = _prep_maps(inputs)
    res = run_bass_kernel_spmd(nc, maps, list(range(NCORE)))
    outs = []
    for c in range(NCORE):
        arr = res.results[c]["out"].reshape(P, CT, BPC)
        outs.append(arr.transpose(2, 1, 0).reshape(BPC, C))
    return np.concatenate(outs, axis=0).astype(np.float32)


if __name__ == "__main__":
    rng = np.random.default_rng(0)
    ins = {
        "x": rng.standard_normal((B, C, 16, 16), dtype=np.float32),
        "pos_emb": rng.standard_normal((C, L), dtype=np.float32) / 32,
        "w_qkv": rng.standard_normal((3 * C, C), dtype=np.float32) / 32,
        "b_qkv": rng.standard_normal((3 * C,), dtype=np.float32) * 0.1,
        "w_c": rng.standard_normal((C, C), dtype=np.float32) / 32,
        "b_c": rng.standard_normal((C,), dtype=np.float32) * 0.1,
    }
    o = kernel(**ins)
    print("out", o.shape, o.dtype, float(np.abs(o).mean()))



# revision 12
# speedup vs baseline: 1.5831x; 1.5831x over previous
"""AttentionPool2d Trainium2 kernel, 8-core batch-data-parallel (v2).

Math (reference returns only query position 0):
  x' = x + pos_sp  (host-folded), posc = pos0 - mean(pos_sp)
  sums = sum_s x'_s ; xf_m = sums/256 + posc
  q0 = (1/8)(W_q xf_m + b_q)            (the only query needed; 1/8 = attn scale^2)
  u_h = W_k_h^T q0_h  (folds W_k into the query; k never materialized)
  lg_sp = u^T x' ; lg_m = rowsum(lg_sp)/256 + u^T posc
  w = softmax([lg_sp | lg_m]) ; w' = w_sp + w_m/256
  y = x' @ w'^T ; a0 = blockdiag(W_v) y + (W_v posc) w_m
  out = w_c a0 + (w_c b_v + b_c)        (bias added on host)

All-batch (b,h)=128 packed layout after the u stage: one softmax, two PE
transposes, wide-moving matmuls. b_k provably drops out (softmax shift).
"""
import sys
sys.path.insert(0, "/opt/trn_rl_repo")
import numpy as np
import ml_dtypes
from contextlib import ExitStack

from concourse import bacc, tile, mybir
import concourse.bass as bass
from concourse import masks
from concourse.bass_utils import run_bass_kernel_spmd

P = 128
B, C, S2, L = 64, 1024, 256, 257
NH = 16
NCORE, BPC, CT = 8, 8, 8          # cores, batches/core, c-tiles (and q-tiles)
F32R = mybir.dt.float32r
F32 = mybir.dt.float32
BF16 = mybir.dt.bfloat16
AF = mybir.ActivationFunctionType
AX = mybir.AxisListType
OP = mybir.AluOpType
SCL = 1.0 / 8.0                    # (1/ch^0.25)^2 folded into q0


def _body(ctx: ExitStack, tc, d):
    nc = tc.nc
    const = ctx.enter_context(tc.tile_pool(name="const", bufs=1))
    xpool = ctx.enter_context(tc.tile_pool(name="xpool", bufs=1))
    wpool = ctx.enter_context(tc.tile_pool(name="wpool", bufs=1))
    work = ctx.enter_context(tc.tile_pool(name="work", bufs=1))
    acc = ctx.enter_context(tc.tile_pool(name="acc", bufs=1))
    psB = ctx.enter_context(tc.tile_pool(name="psB", bufs=4, space="PSUM"))
    psS = ctx.enter_context(tc.tile_pool(name="psS", bufs=2, space="PSUM"))

    # ---- tiles ----
    xn = xpool.tile([P, BPC, CT, L], BF16)          # x' natural + posc col 256
    xtn = xpool.tile([P, BPC, 2, C], BF16)          # x'^T (s-part)
    wqt = wpool.tile([P, CT, C], BF16)              # (1/2048) W_q^T (c-part, q)
    wkn = wpool.tile([P, CT, C], BF16)              # W_k natural (krow-part, c)
    wvt = wpool.tile([P, CT, C], BF16)              # W_v^T (c-part, vch)
    wct = wpool.tile([P, CT, C], BF16)              # w_c^T (vch-part, o)
    qbias = wpool.tile([1, C], BF16)                # (1/8)(W_q posc + b_q)
    vposc = wpool.tile([1, C], BF16)                # W_v posc

    # ---- DMAs in FIFO priority order ----
    nc.sync.dma_start(qbias[:], d["qbias"].ap())
    nc.sync.dma_start(vposc[:], d["vposc"].ap())
    for h in range(4):
        nc.sync.dma_start(xn[:, 2 * h:2 * h + 2], d["xn"].ap()[:, 2 * h:2 * h + 2])
    nc.sync.dma_start(wqt[:], d["wqt"].ap())
    nc.sync.dma_start(wkn[:], d["wkn"].ap())
    for h in range(2):
        nc.sync.dma_start(xtn[:, 4 * h:4 * h + 4], d["xtn"].ap()[:, 4 * h:4 * h + 4])
    nc.sync.dma_start(wvt[:], d["wvt"].ap())
    nc.sync.dma_start(wct[:], d["wct"].ap())

    identf = const.tile([P, P], F32)
    masks.make_identity(nc, identf[:])
    ident = const.tile([P, P], BF16)
    nc.vector.tensor_copy(ident[:], identf[:])
    ones8 = const.tile([1, BPC], BF16)
    nc.vector.memset(ones8[:], 1.0)

    # ---- stage A: sums over s, xf0 ----
    sums = acc.tile([P, BPC, CT], F32R)             # (b, j)
    for h in range(4):
        nc.vector.reduce_sum(sums[:, 2 * h:2 * h + 2, :],
                             xn[:, 2 * h:2 * h + 2, :, 0:S2], axis=AX.X)
    xf0 = acc.tile([P, BPC, CT], BF16)
    nc.vector.tensor_copy(xf0[:], sums[:])

    # ---- q0 (+bias via ones outer-product) ----
    q0p = psS.tile([P, CT, BPC], F32, tag="ps")     # (i, b)
    for i in range(CT):
        for j in range(CT):
            nc.tensor.matmul(q0p[:, i, :], wqt[:, j, i * P:(i + 1) * P],
                             xf0[:, :, j], start=(j == 0), stop=False)
        nc.tensor.matmul(q0p[:, i, :], qbias[0:1, i * P:(i + 1) * P],
                         ones8[:], start=False, stop=True)

    # block-diagonal q0 for the per-head W_k^T fold: col = t*16 + b*2 + h'
    q0blk = acc.tile([P, CT, BPC, 2], BF16)
    nc.vector.memset(q0blk[:], 0.0)
    nc.scalar.activation(q0blk[0:64, :, :, 0], q0p[0:64, :, :], AF.Copy)
    nc.scalar.activation(q0blk[64:P, :, :, 1], q0p[64:P, :, :], AF.Copy)

    # ---- u = blockdiag(W_k)^T q0 ; permuted to (b-major, h) columns ----
    usb = acc.tile([P, CT, BPC, CT, 2], BF16)       # [c-part, j, b, t, h']
    for j in range(CT):
        up = psS.tile([P, CT, BPC, 2], F32, tag="ps")   # (t, b, h')
        for t in range(CT):
            nc.tensor.matmul(up[:, t, :, :], wkn[:, t, j * P:(j + 1) * P],
                             q0blk[:, t, :, :], start=True, stop=True)
        nc.vector.tensor_copy(usb[:, j], up[:].transpose([0, 2, 1, 3]))

    # ---- logits: per-b 16-col stationary into 32-spaced psum blocks ----
    # group g = b//4 holds 4 batches at partition bases 32*(b%4); row =
    # 32*(b%4) + h within a group. Col 256 = u^T posc (posc is x col 256).
    lgps = []
    for g in range(2):
        lgp = psB.tile([P, 512], F32, tag="pb")
        for k in range(4):
            b = g * 4 + k
            for j in range(CT):
                nc.tensor.matmul(lgp[32 * k:32 * k + 16, 0:L],
                                 usb[:, j, b], xn[:, b, j, :],
                                 start=(j == 0), stop=(j == CT - 1),
                                 tile_position=(0, 32 * k))
        lgps.append(lgp)
    lgall = [work.tile([P, L + 3], F32, tag=f"lgall{g}", name=f"lgall{g}")
             for g in range(2)]
    nc.vector.tensor_copy(lgall[0][:, 0:L], lgps[0][:, 0:L])
    nc.scalar.activation(lgall[1][:, 0:L], lgps[1][:, 0:L], AF.Copy)

    # mean-token logit col = rowsum/256 + u^T posc  (then batched softmax)
    wps = []
    wmcols = []
    for g in range(2):
        lg = lgall[g]
        rowsum = work.tile([P, 1], F32, tag=f"rs{g}")
        nc.vector.reduce_sum(rowsum[:], lg[:, 0:S2], axis=AX.X)
        nc.vector.tensor_scalar(lg[:, S2:S2 + 1], rowsum[:], 1.0 / S2,
                                lg[:, S2:S2 + 1], op0=OP.mult, op1=OP.add)
        negmax = work.tile([P, 1], F32, tag=f"nm{g}")
        nc.vector.reduce_max(negmax[:], lg[:, 0:L], axis=AX.X, negate=True)
        ex = work.tile([P, L], F32R, tag=f"ex{g}")
        sumexp = work.tile([P, 1], F32R, tag=f"se{g}")
        nc.scalar.activation(ex[:], lg[:, 0:L], AF.Exp, bias=negmax[:],
                             accum_out=sumexp[:])
        recip = work.tile([P, 1], F32, tag=f"rc{g}")
        nc.vector.reciprocal(recip[:], sumexp[:])
        emdiv = work.tile([P, 1], F32, tag=f"ed{g}")
        nc.vector.tensor_scalar_mul(emdiv[:], ex[:, S2:L], 1.0 / S2)
        wp = work.tile([P, S2], BF16, tag=f"wp{g}")  # w' = (e_sp + e_m/256) r
        nc.vector.tensor_scalar(wp[:], ex[:, 0:S2], emdiv[:], recip[:],
                                op0=OP.add, op1=OP.mult)
        wmcol = work.tile([P, 1], BF16, tag=f"wm{g}")  # w_m = e_m r
        nc.vector.tensor_scalar(wmcol[:], ex[:, S2:L], recip[:], None,
                                op0=OP.mult)
        wps.append(wp)
        wmcols.append(wmcol)

    # ---- transposes: w'^T chunks (s-part, 32-spaced (b,h)) + w_m row ----
    wtas = []
    wmrow = work.tile([1, BPC, 16], BF16)
    for g in range(2):
        wta = acc.tile([P, 2, P], BF16, tag=f"wta{g}")
        for t2 in range(2):
            tp2 = psS.tile([P, P], BF16, tag="ps")
            nc.tensor.transpose(tp2[:], wps[g][:, t2 * P:(t2 + 1) * P],
                                ident[:])
            nc.vector.tensor_copy(wta[:, t2], tp2[:])
        wtas.append(wta)
        wmp = psS.tile([1, 4, 32], BF16, tag="ps")
        nc.tensor.transpose(wmp[0:1], wmcols[g][:], ident[:])
        nc.vector.tensor_copy(wmrow[0:1, g * 4:(g + 1) * 4, :],
                              wmp[0:1, :, 0:16])

    # ---- y^T = w'^T-stationary @ x'^T  -> 32-spaced [(b,h), c] ----
    yTs = [acc.tile([P, C], BF16, tag=f"yT{g}", name=f"yT{g}")
           for g in range(2)]
    for b in range(BPC):
        g, k = b // 4, b % 4
        for half in range(2):
            yp = psB.tile([P, 512], F32, tag="pb")
            for t2 in range(2):
                nc.tensor.matmul(yp[:], wtas[g][:, t2],
                                 xtn[:, b, t2, half * 512:(half + 1) * 512],
                                 start=(t2 == 0), stop=(t2 == 1))
            if b % 2 == 0:
                nc.vector.tensor_copy(
                    yTs[g][32 * k:32 * k + 16, half * 512:(half + 1) * 512],
                    yp[32 * k:32 * k + 16, 0:512])
            else:
                nc.scalar.activation(
                    yTs[g][32 * k:32 * k + 16, half * 512:(half + 1) * 512],
                    yp[32 * k:32 * k + 16, 0:512], AF.Copy)

    # ---- transpose y^T -> y [c-part, (b,h)], compacting 32-spacing ----
    yall = acc.tile([P, CT, BPC, 16], BF16)
    for g in range(2):
        for j in range(CT):
            tp = psS.tile([P, 4, 32], BF16, tag="ps")
            nc.tensor.transpose(tp[:], yTs[g][:, j * P:(j + 1) * P], ident[:])
            nc.vector.tensor_copy(yall[:, j, g * 4:(g + 1) * 4, :],
                                  tp[:, :, 0:16])

    # ---- a0 = blockdiag(W_v) y + vposc * w_m ----
    a0ps = psS.tile([P, CT, BPC, 2], F32, tag="ps")
    for r in range(CT):
        for j in range(CT):
            nc.tensor.matmul(a0ps[:, r, :, :], wvt[:, j, r * P:(r + 1) * P],
                             yall[:, j, :, 2 * r:2 * r + 2],
                             start=(j == 0), stop=False)
        nc.tensor.matmul(a0ps[:, r, :, :], vposc[0:1, r * P:(r + 1) * P],
                         wmrow[0:1, :, 2 * r:2 * r + 2], start=False, stop=True)
    a0blk = acc.tile([P, CT, BPC], BF16)            # (vch-part, (r, b))
    nc.scalar.activation(a0blk[0:64, :, :], a0ps[0:64, :, :, 0], AF.Copy)
    nc.scalar.activation(a0blk[64:P, :, :], a0ps[64:P, :, :, 1], AF.Copy)

    # ---- out^T = a0-stationary @ w_c^T  -> [b, o] ----
    osb = acc.tile([BPC, 2, 512], F32)
    for half in range(2):
        op = psB.tile([P, 512], F32, tag="pb")
        for r in range(CT):
            nc.tensor.matmul(op[0:BPC, :], a0blk[:, r, :],
                             wct[:, r, half * 512:(half + 1) * 512],
                             start=(r == 0), stop=(r == CT - 1))
        nc.vector.tensor_copy(osb[:, half, :], op[0:BPC, :])
    nc.sync.dma_start(d["out"].ap(), osb[:])


_CACHE = {}


def _get_nc():
    if "nc" in _CACHE:
        return _CACHE["nc"]
    nc = bacc.Bacc("TRN2", target_bir_lowering=False, debug=False,
                   num_devices=NCORE)
    d = {}
    d["xn"] = nc.dram_tensor("xn", [P, BPC, CT, L], BF16, kind="ExternalInput")
    d["xtn"] = nc.dram_tensor("xtn", [P, BPC, 2, C], BF16, kind="ExternalInput")
    d["wqt"] = nc.dram_tensor("wqt", [P, CT, C], BF16, kind="ExternalInput")
    d["wkn"] = nc.dram_tensor("wkn", [P, CT, C], BF16, kind="ExternalInput")
    d["wvt"] = nc.dram_tensor("wvt", [P, CT, C], BF16, kind="ExternalInput")
    d["wct"] = nc.dram_tensor("wct", [P, CT, C], BF16, kind="ExternalInput")
    d["qbias"] = nc.dram_tensor("qbias", [1, C], BF16, kind="ExternalInput")
    d["vposc"] = nc.dram_tensor("vposc", [1, C], BF16, kind="ExternalInput")
    d["out"] = nc.dram_tensor("out", [BPC, 2, 512], F32, kind="ExternalOutput")
    with tile.TileContext(nc) as tc, ExitStack() as ctx, \
            nc.allow_low_precision(reason="float32r tiles hold f32 bits"):
        _body(ctx, tc, d)
    nc.compile()
    _CACHE["nc"] = nc
    return nc


def _prep_maps(inputs):
    xf = inputs["x"].reshape(B, C, S2).astype(np.float32)
    pos = inputs["pos_emb"].astype(np.float32)
    xp = xf + pos[None, :, 1:]
    posc = pos[:, 0] - pos[:, 1:].mean(axis=1)          # [C]
    wqkv = inputs["w_qkv"].astype(np.float32)
    wq, wk, wv = wqkv[0:C], wqkv[C:2 * C], wqkv[2 * C:3 * C]
    bq = inputs["b_qkv"][0:C].astype(np.float32)
    bv = inputs["b_qkv"][2 * C:3 * C].astype(np.float32)
    wc = inputs["w_c"].astype(np.float32)
    bc = inputs["b_c"].astype(np.float32)

    bf = ml_dtypes.bfloat16
    wqt = np.ascontiguousarray(
        (wq.T * (SCL / S2)).reshape(CT, P, C).transpose(1, 0, 2)).astype(bf)
    wkn = np.ascontiguousarray(
        wk.reshape(CT, P, C).transpose(1, 0, 2)).astype(bf)
    wvt = np.ascontiguousarray(
        wv.T.reshape(CT, P, C).transpose(1, 0, 2)).astype(bf)
    wct = np.ascontiguousarray(
        wc.T.reshape(CT, P, C).transpose(1, 0, 2)).astype(bf)
    qbias = np.ascontiguousarray((SCL * (wq @ posc + bq))[None, :]).astype(bf)
    vposc = np.ascontiguousarray((wv @ posc)[None, :]).astype(bf)
    poscp = posc.reshape(CT, P).T                        # [P, CT]
    shared = dict(wqt=wqt, wkn=wkn, wvt=wvt, wct=wct,
                  qbias=qbias, vposc=vposc)
    maps = []
    for c in range(NCORE):
        xc = xp[c * BPC:(c + 1) * BPC]                   # [8, 1024, 256]
        m = dict(shared)
        xnc = np.empty((P, BPC, CT, L), dtype=np.float32)
        xnc[:, :, :, 0:S2] = xc.reshape(BPC, CT, P, S2).transpose(2, 0, 1, 3)
        xnc[:, :, :, S2] = poscp[:, None, :]
        m["xn"] = np.ascontiguousarray(xnc).astype(bf)
        m["xtn"] = np.ascontiguousarray(
            xc.reshape(BPC, C, 2, P).transpose(3, 0, 2, 1)).astype(bf)
        maps.append(m)
    _CACHE["hostbias"] = wc @ bv + bc                    # [C]
    return maps


def kernel(**inputs) -> np.ndarray:
    nc = _get_nc()
    maps = _prep_maps(inputs)
    res = run_bass_kernel_spmd(nc, maps, list(range(NCORE)))
    hb = _CACHE["hostbias"]
    outs = []
    for c in range(NCORE):
        arr = res.results[c]["out"].reshape(BPC, C).astype(np.float32)
        outs.append(arr + hb[None, :])
    return np.concatenate(outs, axis=0)


if __name__ == "__main__":
    rng = np.random.default_rng(0)
    ins = {
        "x": rng.standard_normal((B, C, 16, 16), dtype=np.float32),
        "pos_emb": rng.standard_normal((C, L), dtype=np.float32) / 32,
        "w_qkv": rng.standard_normal((3 * C, C), dtype=np.float32) / 32,
        "b_qkv": rng.standard_normal((3 * C,), dtype=np.float32) * 0.1,
        "w_c": rng.standard_normal((C, C), dtype=np.float32) / 32,
        "b_c": rng.standard_normal((C,), dtype=np.float32) * 0.1,
    }
    o = kernel(**ins)
    print("out", o.shape, o.dtype, float(np.abs(o).mean()))
